# revision 27
# baseline (speedup 1.0000x reference)
"""Trainium2 Bass kernel for nn_AttentionRegression (ragged segment attention).

Math reformulation (exact):
  y[b] = g_x*f_x[b] + g_b + num[b]/den[b]
    w_t   = n_t . g_n                     (g weights applied per neighbour row)
    z_t   = exp(sigmoid(tanh(n_t @ W1n^T + f_x[seg]*w1x + b1) @ W2 + b2))
    num_b = sum_{t in seg b} z_t * w_t ;  den_b = sum z_t
  (softmax max-subtraction dropped: scores are sigmoid outputs in (0,1), so
   exp() is stable and the ratio is mathematically unchanged.)

Device layout: segments sorted by length into 16 strata; stratum k supplies one
128-segment block to each of the 8 cores, padded to a common length Ls[k]
(multiple of 8). Neighbours are shipped transposed+bf16 as nt[128 feat, col]
with col = blockbase + pos*128 + seg_local, so the per-row scalars computed by
the PE land as [seg=partition, pos=free] and segment sums are free-dim reduces.

Per 128-row tile the PE does LDWEIGHTS(nt tile) + matmul against a [128,13]
weight block (12 cols = W1n^T, col 12 = g_n). The per-sample bias fx*w1x + b1
is added on the DVE (per-block [128,12] bias tile, stride-0 broadcast over
positions) so the PE queue carries nothing but the position matmuls — the
old per-chunk rank-1 bias matmul cost ~12us of real PE time (416 cols each),
and LDWEIGHTS engine time is unmodeled in the cost model (TODO in
instruction_cost_v2.rs), so on silicon the PE is the binding engine and this
relief is a real win even though TimelineSim scores it a wash.

Modeled ~108us/core against a ~98us DMA floor (34.7MB bf16 @358GB/s);
the neighbour stream is gapless, alternating between the SP and gpsimd DGE
queues (dual_dma). The last block's position chunks taper down and its
softmax epilogue is split so only the final taper chunk's compute trails the
last DMA. fp8 was evaluated and rejected: e4m3 neighbours push end-to-end
rel err to 1.8e-2 against the 2e-2 gate (w_t = n.g is the critical path).
"""

import numpy as np
import ml_dtypes
from contextlib import ExitStack

import concourse.bass as bass
import concourse.bacc as bacc
import concourse.tile as tile
from concourse import mybir
from concourse.bass_utils import run_bass_kernel_spmd

B, T, NF, H = 16384, 1048576, 128, 12
NCORES = 8
SEGS_PER_BLOCK = 128
CH = 32  # positions per superchunk (psum [128, 13*CH])
F32 = mybir.dt.float32
BF16 = mybir.dt.bfloat16
AL = mybir.AluOpType
AF = mybir.ActivationFunctionType

_program_cache = {}


def build_program(Ls, nblk, nrep=1, dual_dma=False, ch=CH, bufs_big=4,
                  bufs_ps=4, bufs_hp=3, taper=(10, 10)):
    nc = bacc.Bacc(
        "TRN2",
        target_bir_lowering=False,
        debug=False,
        enable_asserts=False,
    )
    sumL = sum(Ls)
    R = 128 * sumL
    nt = nc.dram_tensor("nt", [128, R], BF16, kind="ExternalInput").ap()
    w13 = nc.dram_tensor("w13", [128, 13], BF16, kind="ExternalInput").ap()
    w2rep = nc.dram_tensor("w2rep", [128, ch * H], BF16, kind="ExternalInput").ap()
    # aux3 cols: 0 b2/2 | 1 gx | 2 gb
    aux3 = nc.dram_tensor("aux3", [128, 3], F32, kind="ExternalInput").ap()
    fxd = nc.dram_tensor("fx", [128, nblk], F32, kind="ExternalInput").ap()
    # wb cols 0:12 = w1x (bias weight on fx), 12:24 = b1; rows identical
    wbd = nc.dram_tensor("wb", [128, 2 * H], F32, kind="ExternalInput").ap()
    maskd = nc.dram_tensor("mask", [128, sumL], BF16, kind="ExternalInput").ap()
    yd = nc.dram_tensor("y", [128, nblk], F32, kind="ExternalOutput").ap()

    with tile.TileContext(nc) as tc, ExitStack() as ctx:
        if nrep > 1:
            ctx.enter_context(tc.For_i(0, nrep, 1, name="bench"))
        singles = ctx.enter_context(tc.tile_pool(name="singles", bufs=1))
        bigp = ctx.enter_context(tc.tile_pool(name="bigp", bufs=bufs_big))
        psp = ctx.enter_context(tc.tile_pool(name="psp", bufs=bufs_ps, space="PSUM"))
        hp = ctx.enter_context(tc.tile_pool(name="hp", bufs=bufs_hp))

        # small loads ride the gpsimd SWDGE queue so the SP queue can start
        # streaming neighbour blocks immediately
        w13_s = singles.tile([128, 13], BF16)
        nc.gpsimd.dma_start(out=w13_s[:], in_=w13)
        w2rep_s = singles.tile([128, ch * H], BF16)
        nc.gpsimd.dma_start(out=w2rep_s[:], in_=w2rep)
        aux3_s = singles.tile([128, 3], F32)
        nc.gpsimd.dma_start(out=aux3_s[:], in_=aux3)
        fx_s = singles.tile([128, nblk], F32)
        nc.gpsimd.dma_start(out=fx_s[:], in_=fxd)
        wb_s = singles.tile([128, 2 * H], F32)
        nc.gpsimd.dma_start(out=wb_s[:], in_=wbd)
        mask_s = singles.tile([128, sumL], BF16)
        nc.gpsimd.dma_start(out=mask_s[:], in_=maskd)

        s_all = singles.tile([128, sumL], F32)
        w_all = singles.tile([128, sumL], F32)
        den_all = singles.tile([128, nblk], F32)
        num_all = singles.tile([128, nblk], F32)
        den2 = singles.tile([128, 2], F32)
        num2 = singles.tile([128, 2], F32)

        def epilogue(e0, elen, dcol, ncol):
            # softmax-sum epilogue, fully inside the {Tanh, Exp, Copy} set:
            # sigmoid(x) = 0.5 + 0.5*tanh(x/2) and softmax drops constants, so
            # z = exp(0.5*tanh(0.5*(s + b2))) has the exact softmax ratios.
            u = hp.tile([128, elen], F32, tag="u")
            nc.scalar.activation(out=u[:], in_=s_all[:, e0: e0 + elen],
                                 func=AF.Tanh, bias=aux3_s[:, 0:1], scale=0.5)
            z = hp.tile([128, elen], F32, tag="z")
            nc.scalar.activation(out=z[:], in_=u[:], func=AF.Exp, scale=0.5)
            zm = hp.tile([128, elen], F32, tag="zm")
            nc.vector.tensor_mul(zm[:], z[:], mask_s[:, e0: e0 + elen])
            zw = hp.tile([128, elen], F32, tag="zw")
            nc.vector.tensor_mul(zw[:], zm[:], w_all[:, e0: e0 + elen])
            nc.vector.reduce_sum(out=dcol, in_=zm[:],
                                 axis=mybir.AxisListType.X)
            nc.vector.reduce_sum(out=ncol, in_=zw[:],
                                 axis=mybir.AxisListType.X)

        def chunk_sizes(L, last_block):
            """Chunk positions; on the last block taper the final chunks so
            the post-DMA compute drain is short."""
            if not last_block:
                return [min(ch, L - p) for p in range(0, L, ch)]
            tail = []
            rem = L
            for t in taper:
                if rem - t <= 0:
                    break
                tail.append(t)
                rem -= t
            head = [min(ch, rem - p) for p in range(0, rem, ch)] if rem else []
            return head + tail

        col = 0
        soff = 0
        nchunk = 0
        for g in range(nblk):
            L = Ls[g]
            last = g == nblk - 1
            sizes = chunk_sizes(L, last)
            splitA = L - sizes[-1] if (last and len(sizes) > 1) else None
            # per-block tanh bias: bias_g[p, j] = fx[p, g]*w1x[j] + b1[j].
            # Adding it on the DVE (broadcast over positions) keeps the PE
            # queue free of the old per-chunk rank-1 bias matmul, which cost
            # ~12us of PE sequencer occupancy across the program.
            bias_g = hp.tile([128, H], F32, tag="bias")
            nc.vector.scalar_tensor_tensor(
                out=bias_g[:], in0=wb_s[:, 0:H], scalar=fx_s[:, g:g + 1],
                in1=wb_s[:, H:2 * H], op0=AL.mult, op1=AL.add)
            p0 = 0
            for c in sizes:
                assert p0 + c <= L
                ntb = bigp.tile([128, 128 * c], BF16, tag="ntb")
                eng = nc.gpsimd if (dual_dma and nchunk % 2) else nc.sync
                nchunk += 1
                eng.dma_start(
                    out=ntb[:],
                    in_=nt[:, col + p0 * 128: col + (p0 + c) * 128])
                ps = psp.tile([128, 13 * c], F32, tag="ps")
                for i in range(c):
                    # each position writes a disjoint 13-col psum region, so
                    # every matmul can start=True (resets only its own region)
                    # while one stop=True closes the whole chunk's group —
                    # a single sem update instead of one per position.
                    nc.tensor.matmul(
                        ps[:, 13 * i: 13 * (i + 1)],
                        lhsT=ntb[:, i * 128: (i + 1) * 128],
                        rhs=w13_s[:], start=True, stop=(i == c - 1),
                        skip_group_check=True)
                psv = ps[:].rearrange("p (c t) -> p c t", t=13)
                ti = hp.tile([128, c * H], BF16, tag="ti")
                nc.vector.tensor_add(
                    ti[:].rearrange("p (c t) -> p c t", t=H),
                    psv[:, :, 0:12],
                    bias_g[:].unsqueeze(1).broadcast_to([128, c, H]))
                th = hp.tile([128, c * H], BF16, tag="th")
                nc.scalar.activation(
                    out=th[:].rearrange("p (c t) -> p c t", t=H),
                    in_=ti[:].rearrange("p (c t) -> p c t", t=H), func=AF.Tanh)
                m = hp.tile([128, c * H], BF16, tag="m")
                nc.vector.tensor_mul(m[:], th[:], w2rep_s[:, 0: c * H])
                nc.vector.reduce_sum(
                    out=s_all[:, soff + p0: soff + p0 + c],
                    in_=m[:].rearrange("p (c t) -> p c t", t=H),
                    axis=mybir.AxisListType.X)
                nc.scalar.activation(
                    out=w_all[:, soff + p0: soff + p0 + c],
                    in_=psv[:, :, 12], func=AF.Copy)
                p0 += c
                if splitA is not None and p0 == splitA:
                    # last block: drain most of the epilogue early so only
                    # the final taper chunk's work trails the last DMA
                    epilogue(soff, splitA, den2[:, 0:1], num2[:, 0:1])

            if splitA is not None:
                epilogue(soff + splitA, L - splitA, den2[:, 1:2], num2[:, 1:2])
                nc.vector.reduce_sum(out=den_all[:, g:g + 1], in_=den2[:],
                                     axis=mybir.AxisListType.X)
                nc.vector.reduce_sum(out=num_all[:, g:g + 1], in_=num2[:],
                                     axis=mybir.AxisListType.X)
            else:
                epilogue(soff, L, den_all[:, g:g + 1], num_all[:, g:g + 1])
            col += 128 * L
            soff += L

        den_eps = singles.tile([128, nblk], F32)
        nc.vector.tensor_scalar(
            out=den_eps[:], in0=den_all[:], scalar1=1e-30, scalar2=None,
            op0=AL.add)
        rec_all = singles.tile([128, nblk], F32)
        nc.vector.reciprocal(out=rec_all[:], in_=den_eps[:])
        t_all = singles.tile([128, nblk], F32)
        nc.vector.tensor_mul(t_all[:], num_all[:], rec_all[:])
        y1_all = singles.tile([128, nblk], F32)
        nc.vector.scalar_tensor_tensor(
            out=y1_all[:], in0=fx_s[:], scalar=aux3_s[:, 1:2], in1=t_all[:],
            op0=AL.mult, op1=AL.add)
        y_all = singles.tile([128, nblk], F32)
        nc.vector.tensor_scalar(
            out=y_all[:], in0=y1_all[:], scalar1=aux3_s[:, 2:3], scalar2=None,
            op0=AL.add)
        nc.sync.dma_start(out=yd, in_=y_all[:])
    nc.compile()
    return nc, R


def prep_host(f_x, neighbours, seg_ids, f_W1, f_b1, f_W2, f_b2, g_W, g_b):
    """Shard/pack inputs. Returns (Ls, nblk, in_maps, order)."""
    lens_all = np.bincount(seg_ids, minlength=B).astype(np.int64)
    order = np.argsort(-lens_all, kind="stable")
    nblk = B // (SEGS_PER_BLOCK * NCORES)  # 16
    stratum = SEGS_PER_BLOCK * NCORES  # 1024
    Ls = []
    for k in range(nblk):
        m = int(lens_all[order[k * stratum:(k + 1) * stratum]].max())
        Ls.append(max(1, m))
    sumL = sum(Ls)
    R = 128 * sumL

    row_start = np.zeros(B + 1, np.int64)
    row_start[1:] = np.cumsum(lens_all)
    nbf = neighbours.astype(ml_dtypes.bfloat16)

    w13 = np.zeros((128, 13), np.float32)
    w13[:, 0:12] = f_W1[:, 1:].T
    w13[:, 12] = g_W[0, 1:]
    w13 = w13.astype(ml_dtypes.bfloat16)

    w1x = f_W1[:, 0].astype(np.float32)
    w2rep = np.tile(np.concatenate([f_W2[0], ]).astype(np.float32), CH)
    w2rep = np.tile(w2rep[None, :], (128, 1)).astype(ml_dtypes.bfloat16)

    aux3 = np.zeros((128, 3), np.float32)
    aux3[:, 0] = 0.5 * f_b2[0]
    aux3[:, 1] = g_W[0, 0]
    aux3[:, 2] = g_b[0]

    wb = np.empty((128, 2 * H), np.float32)
    wb[:, 0:H] = w1x[None, :]
    wb[:, H:2 * H] = f_b1.astype(np.float32)[None, :]

    in_maps = []
    for c in range(NCORES):
        idx = np.empty(R, np.int64)
        valid = np.empty(R, bool)
        fx_mat = np.empty((128, nblk), np.float32)
        mask = np.empty((128, sumL), ml_dtypes.bfloat16)
        off = 0
        soff = 0
        for g in range(nblk):
            Lg = Ls[g]
            gids = order[g * stratum + 128 * c: g * stratum + 128 * (c + 1)]
            pos = np.arange(Lg)[:, None]
            rows = row_start[gids][None, :] + pos          # [Lg, 128]
            val = pos < lens_all[gids][None, :]
            blockn = Lg * 128
            idx[off:off + blockn] = np.where(val, rows, 0).reshape(-1)
            valid[off:off + blockn] = val.reshape(-1)
            fx_mat[:, g] = f_x[gids, 0]
            mask[:, soff:soff + Lg] = val.T.astype(ml_dtypes.bfloat16)
            off += blockn
            soff += Lg
        nrows = nbf[idx]                                   # [R, 128] bf16
        nrows[~valid] = ml_dtypes.bfloat16(0)
        nt_c = np.ascontiguousarray(nrows.T)               # [128, R]
        in_maps.append({
            "nt": nt_c, "w13": w13, "w2rep": w2rep, "aux3": aux3,
            "fx": fx_mat, "wb": wb, "mask": mask,
        })
    return Ls, nblk, in_maps, order


def assemble_output(results, order, nblk):
    stratum = SEGS_PER_BLOCK * NCORES
    y_full = np.empty(B, np.float32)
    for c in range(NCORES):
        yc = results[c]["y"]  # [128, nblk]
        for g in range(nblk):
            y_full[order[g * stratum + 128 * c: g * stratum + 128 * (c + 1)]] = yc[:, g]
    return y_full[:, None]


def kernel(**inputs) -> np.ndarray:
    args = {k: np.asarray(v) for k, v in inputs.items()}
    Ls, nblk, in_maps, order = prep_host(
        args["f_x"], args["neighbours"], args["seg_ids"],
        args["f_W1"], args["f_b1"], args["f_W2"], args["f_b2"],
        args["g_W"], args["g_b"])
    key = (tuple(Ls), nblk)
    if key not in _program_cache:
        _program_cache[key] = build_program(Ls, nblk, dual_dma=True,
                                            bufs_big=6, bufs_ps=6, bufs_hp=4)
    nc, _ = _program_cache[key]
    res = run_bass_kernel_spmd(nc, in_maps, core_ids=list(range(NCORES)))
    return assemble_output(res.results, order, nblk)

